# revision 6
# baseline (speedup 1.0000x reference)
"""Multi-head attention (B=4, S=2048, D=512, H=8) on 8 trn2 cores.

Sharding: core c handles batch b=c//2 and the head-quad qh=c%2 (heads
4*qh..4*qh+3, i.e. 2 head-PAIRS). The kernel is restructured around keeping
the Scalar (ACT) engine -- which does the softmax exp and is the true
bottleneck at 1 elem/cycle/lane @1.2GHz (~147us/core) -- saturated:

- Scores for a head PAIR run concurrently on the PE via row tiling
  (tile_position (0,0)/(64,0), K=64 each): both heads' scores for one
  128-key chunk land in one [128, 1024] psum tile in ~512 cycles, and a
  single N=1024 exp call covers the pair.
- Units are (query-block of 512, pair); sp is double-buffered so the PE
  writes scores for chunk j+1 while ACT exps chunk j; attn@v (with the
  ones-column denominator trick) drains at bf16 from SBUF behind exp.
- All projection / output-projection work is interleaved into the PE's
  slack inside the jc loops (useful filler instead of the old HAM-bridge
  dummies), and per-chunk kT/qT/vv tiles let the first exp start ~8us in.

All matmuls in float32r (1 cycle/row). Softmax skips max-subtraction
(|s| < ~55 whp, exp fits fp32/bf16) exactly like the reference within tol.
"""
import sys

sys.path.insert(0, "/opt/trn_rl_repo")
import numpy as np

B, S, D, H, HD = 4, 2048, 512, 8, 64
HPC = 4          # heads per core
DQ = HPC * HD    # 256 projection dims per core
NCORES = 8
VW = HD + 1      # v block width incl. ones column (65)
QB = 512         # query block
NQB = S // QB    # 4
NJC = S // 128   # 16 key chunks

_cache = {}


def _build_nc():
    import concourse.bacc as bacc
    import concourse.mybir as mybir
    import concourse.tile as tile

    F32, F32R = mybir.dt.float32, mybir.dt.float32r
    BF16 = mybir.dt.bfloat16
    EXP = mybir.ActivationFunctionType.Exp

    nc = bacc.Bacc("TRN2", target_bir_lowering=False, debug=False)

    xT = nc.dram_tensor("xT", [D, S], F32R, kind="ExternalInput")
    wqT = nc.dram_tensor("wqT", [D, DQ], F32R, kind="ExternalInput")
    wkT = nc.dram_tensor("wkT", [D, DQ], F32R, kind="ExternalInput")
    wvT = nc.dram_tensor("wvT", [D, DQ], F32R, kind="ExternalInput")
    woT = nc.dram_tensor("woT", [DQ, D], F32R, kind="ExternalInput")
    outT = nc.dram_tensor("outT", [D, S], F32, kind="ExternalOutput")
    NU = 2 * NQB * 2  # (pair, qb, head) rows
    scr_sums = nc.dram_tensor("scr_sums", [NU, QB], F32)
    scr_recip = nc.dram_tensor("scr_recip", [NU, QB], F32)

    with tile.TileContext(nc) as tc:
        with tc.tile_pool(name="sb", bufs=1) as sb:
            psum = tc.tile_pool(name="psum", bufs=1, space="PSUM")
            pp = psum.__enter__()

            # ---- input DMAs, priority order for earliest first-exp ----
            wk = []
            for d in range(4):
                t = sb.tile([128, DQ], F32R, tag=f"wk{d}", name=f"wk{d}")
                nc.sync.dma_start(out=t[:], in_=wkT[128 * d:128 * (d + 1), :])
                wk.append(t)
            xt = [sb.tile([128, S], F32R, tag=f"xt{d}", name=f"xt{d}")
                  for d in range(4)]
            for d in range(4):
                nc.sync.dma_start(out=xt[d][:, 0:512], in_=xT[128 * d:128 * (d + 1), 0:512])
            wq, wv = [], []
            for nm, dram, lst in (("wq", wqT, wq), ("wv", wvT, wv)):
                for d in range(4):
                    t = sb.tile([128, DQ], F32R, tag=f"{nm}{d}", name=f"{nm}{d}")
                    nc.sync.dma_start(out=t[:], in_=dram[128 * d:128 * (d + 1), :])
                    lst.append(t)
            for sc in range(1, 4):
                for d in range(4):
                    nc.sync.dma_start(
                        out=xt[d][:, sc * 512:(sc + 1) * 512],
                        in_=xT[128 * d:128 * (d + 1), sc * 512:(sc + 1) * 512])
            wo = []
            for kc in range(4):
                t = sb.tile([64, D], F32R, tag=f"wo{kc}", name=f"wo{kc}")
                nc.sync.dma_start(out=t[:], in_=woT[64 * kc:64 * (kc + 1), :])
                wo.append(t)

            # ---- ACT table pre-load: tiny exp at t0 (hidden under DMA) ----
            dumm = sb.tile([128, 8], F32, tag="dumm", name="dumm")
            nc.vector.memset(dumm[:], 0.0)
            dumo = sb.tile([128, 8], F32, tag="dumo", name="dumo")
            nc.scalar.activation(dumo[:], dumm[:], EXP)

            # ---- persistent tiles ----
            # kT/qT per (pair, chunk): partitions 0-63 = head 2p, 64-127 = 2p+1
            kTt = [[sb.tile([128, 512], F32R, tag=f"kT{p}_{sc}", name=f"kT{p}_{sc}")
                    for sc in range(4)] for p in range(2)]
            qTt = [[sb.tile([128, 512], F32R, tag=f"qT{p}_{qb}", name=f"qT{p}_{qb}")
                    for qb in range(NQB)] for p in range(2)]
            vvt = [sb.tile([128, HPC * VW], BF16, tag=f"vv{jc}", name=f"vv{jc}")
                   for jc in range(NJC)]
            oTn = [[sb.tile([64, QB], F32R, tag=f"oTn{h}_{qb}", name=f"oTn{h}_{qb}")
                    for qb in range(NQB)] for h in range(HPC)]
            ones32 = sb.tile([128, 1], F32, tag="ones32", name="ones32")
            nc.vector.memset(ones32[:], 1.0)

            # ---- proj group emitters (each ~1-2k PE cycles + a DVE copy) ----
            def kq_group(dst, wsb, p, col0):
                # dst[:, :] = (w pair-slice).T @ x[:, col0:col0+512]
                ps = pp.tile([128, 512], F32, tag="pg", bufs=2, name="ps")
                for d in range(4):
                    nc.tensor.matmul(
                        ps[:], wsb[d][:, p * 128:(p + 1) * 128],
                        xt[d][:, col0:col0 + 512],
                        start=(d == 0), stop=(d == 3))
                nc.vector.tensor_copy(out=dst[:], in_=ps[:])

            def vv_group(jc):
                psv = pp.tile([128, 512], F32, tag="pg", bufs=2, name="psv")
                for d in range(4):
                    nc.tensor.matmul(
                        psv[:, 0:DQ], xt[d][:, jc * 128:(jc + 1) * 128],
                        wv[d][:, :], start=(d == 0), stop=(d == 3))
                vv_ones = vvt[jc][:, :].rearrange(
                    "p (g w) -> p g w", w=VW)[:, :, HD:HD + 1]
                nc.vector.tensor_copy(
                    out=vv_ones, in_=ones32[:].to_broadcast((128, HPC, 1)))
                for h in range(HPC):
                    nc.vector.tensor_copy(
                        out=vvt[jc][:, VW * h: VW * h + HD],
                        in_=psv[:, HD * h: HD * (h + 1)])

            def outproj_group(m, qb):
                # outT[m-chunk, qb-block] = sum_h woT[h-dims, m-chunk].T @ oTn
                po = pp.tile([128, 512], F32, tag="pg", bufs=2, name="po")
                for kc in range(4):
                    nc.tensor.matmul(
                        po[:], wo[kc][:, m * 128:(m + 1) * 128],
                        oTn[kc][qb][:], start=(kc == 0), stop=(kc == 3))
                ob = sb.tile([128, 512], F32, tag="ob", bufs=4, name="ob")
                nc.vector.tensor_copy(out=ob[:], in_=po[:])
                nc.sync.dma_start(
                    out=outT[m * 128:(m + 1) * 128, qb * QB:(qb + 1) * QB],
                    in_=ob[:])

            # ---- explicit filler schedule: unit u = qb*2 + p; slots[u][jc]
            # is a list of closures emitted right after attnv(jc-1), i.e. in
            # the PE's ACT-bound slack.  Placement respects (a) emission
            # before consumption, (b) input-DMA landing times (so a stalled
            # filler never blocks the FIFO PE queue ahead of scores).
            slots = [dict() for _ in range(2 * NQB)]

            def put(u, jc, fn):
                slots[u].setdefault(jc, []).append(fn)

            # unit 0 = (qb0, p0): stream in remaining vv + kT chunks
            for k in range(2, NJC):
                put(0, k - 2, lambda jc=k: vv_group(jc))
            put(0, 3, lambda: kq_group(kTt[0][1], wk, 0, 512))
            put(0, 5, lambda: kq_group(kTt[0][2], wk, 0, 1024))
            put(0, 9, lambda: kq_group(kTt[0][3], wk, 0, 1536))
            put(0, 11, lambda: kq_group(kTt[1][0], wk, 1, 0))
            put(0, 13, lambda: kq_group(kTt[1][1], wk, 1, 512))
            put(0, 15, lambda: kq_group(qTt[1][0], wq, 1, 0))
            # unit 1 = (qb0, p1): its own late kT chunks + next qT
            put(1, 1, lambda: kq_group(kTt[1][2], wk, 1, 1024))
            put(1, 3, lambda: kq_group(kTt[1][3], wk, 1, 1536))
            put(1, 9, lambda: kq_group(qTt[0][1], wq, 0, 512))
            for qb in range(1, NQB):
                u0 = 2 * qb
                for m in range(4):  # outproj of the previous qblock
                    put(u0, 2 * m + 1, lambda m=m, qb=qb: outproj_group(m, qb - 1))
                put(u0, 9, lambda qb=qb: kq_group(qTt[1][qb], wq, 1, qb * 512))
                if qb + 1 < NQB:
                    put(u0 + 1, 9, lambda qb=qb: kq_group(
                        qTt[0][qb + 1], wq, 0, (qb + 1) * 512))

            # ---- phase P: minimal prefix before unit (qb0, p0) ----
            with nc.named_scope("proj"):
                kq_group(kTt[0][0], wk, 0, 0)
                kq_group(qTt[0][0], wq, 0, 0)
                vv_group(0)
                vv_group(1)

            # ---- attention units ----
            def epilogue(p, qb, e, op_t):
                h = 2 * p + e
                r = (p * NQB + qb) * 2 + e
                otu = sb.tile([65, QB], F32, tag="otu", bufs=2, name="otu")
                nc.vector.tensor_copy(out=otu[:], in_=op_t[:])
                nc.sync.dma_start(out=scr_sums[r:r + 1, :], in_=otu[64:65, :])
                sumsT = sb.tile([128, 4], F32, tag="sumsT", bufs=2, name="sumsT")
                nc.sync.dma_start(
                    out=sumsT[:],
                    in_=scr_sums[r:r + 1, :].rearrange("o (c p) -> (o p) c", p=128))
                recipT = sb.tile([128, 4], F32, tag="recipT", bufs=2, name="recipT")
                nc.vector.reciprocal(recipT[:], sumsT[:])
                nc.sync.dma_start(
                    out=scr_recip[r:r + 1, :].rearrange("o (c p) -> (o p) c", p=128),
                    in_=recipT[:])
                rb = sb.tile([64, QB], F32, tag="rb", bufs=2, name="rb")
                nc.sync.dma_start(
                    out=rb[:], in_=scr_recip[r:r + 1, :].to_broadcast((64, QB)))
                nc.vector.tensor_mul(
                    out=oTn[h][qb][:], in0=otu[0:64, :], in1=rb[:])

            with nc.named_scope("attn"):
                for u in range(2 * NQB):
                    qb, p = u // 2, u % 2
                    op0 = pp.tile([65, QB], F32, tag="op0", bufs=1, name="op0")
                    op1 = pp.tile([65, QB], F32, tag="op1", bufs=1, name="op1")
                    ats = [None] * 16
                    # software-pipelined: attnv lags scores/exp by one jc so
                    # the FIFO PE queue never parks scores behind an
                    # exp-dependent matmul.
                    for jc in range(16):
                        sp = pp.tile([128, 1024], F32, tag="sp", bufs=2,
                                     name="sp")
                        sc, c0 = jc // 4, (jc % 4) * 128
                        for e in range(2):
                            nc.tensor.matmul(
                                sp[:, e * 512:(e + 1) * 512],
                                kTt[p][sc][64 * e:64 * e + 64, c0:c0 + 128],
                                qTt[p][qb][64 * e:64 * e + 64, :],
                                start=True, stop=True,
                                tile_position=(64 * e, 0))
                        at = sb.tile([128, 1024], BF16, tag="at", bufs=3,
                                     name="at")
                        nc.scalar.activation(at[:], sp[:], EXP)
                        ats[jc] = at

                        def attnv(j):
                            for e, op_t in ((0, op0), (1, op1)):
                                nc.tensor.matmul(
                                    op_t[:],
                                    vvt[j][:, VW * (2 * p + e):
                                           VW * (2 * p + e) + VW],
                                    ats[j][:, e * 512:(e + 1) * 512],
                                    start=(j == 0), stop=(j == 15))
                        if jc > 0:
                            attnv(jc - 1)
                        for fn in slots[u].get(jc, ()):
                            fn()
                    attnv(15)
                    for e, op_t in ((0, op0), (1, op1)):
                        epilogue(p, qb, e, op_t)

            with nc.named_scope("outproj"):
                for m in range(4):
                    outproj_group(m, NQB - 1)

            psum.__exit__(None, None, None)

    nc.compile()
    return nc


def _get_nc():
    if "nc" not in _cache:
        _cache["nc"] = _build_nc()
    return _cache["nc"]


def _in_maps(x, w_qkv, w_out):
    x = np.asarray(x, dtype=np.float32)
    w_qkv = np.asarray(w_qkv, dtype=np.float32)
    w_out = np.asarray(w_out, dtype=np.float32)
    maps = []
    for c in range(NCORES):
        b, qh = c // 2, c % 2
        r0 = qh * DQ
        maps.append({
            "xT": np.ascontiguousarray(x[b].T),
            "wqT": np.ascontiguousarray(w_qkv[r0:r0 + DQ].T),
            "wkT": np.ascontiguousarray(w_qkv[D + r0:D + r0 + DQ].T),
            "wvT": np.ascontiguousarray(w_qkv[2 * D + r0:2 * D + r0 + DQ].T),
            "woT": np.ascontiguousarray(w_out[:, r0:r0 + DQ].T),
        })
    return maps


def _gather(results):
    out = np.empty((B, S, D), np.float32)
    for b in range(B):
        acc = results[2 * b]["outT"] + results[2 * b + 1]["outT"]
        out[b] = acc.T
    return out


def run(x, w_qkv, w_out, trace=False):
    from concourse.bass_utils import run_bass_kernel_spmd

    nc = _get_nc()
    res = run_bass_kernel_spmd(
        nc, _in_maps(x, w_qkv, w_out), core_ids=list(range(NCORES)), trace=trace,
    )
    return _gather(res.results), res


def kernel(x, w_qkv, w_out):
    out, _ = run(x, w_qkv, w_out)
    return out


# revision 13
# speedup vs baseline: 1.3018x; 1.3018x over previous
"""Multi-head attention (B=4, S=2048, D=512, H=8) on 8 trn2 cores.

Sharding: core c handles batch b=c//2 and the head-quad qh=c%2 (heads
4*qh..4*qh+3, i.e. 2 head-PAIRS). The kernel is restructured around keeping
the Scalar (ACT) engine -- which does the softmax exp and is the true
bottleneck at 1 elem/cycle/lane @1.2GHz (~147us/core) -- saturated:

- Scores for a head PAIR run concurrently on the PE via row tiling
  (tile_position (0,0)/(64,0), K=64 each): both heads' scores for one
  128-key chunk land in one [128, 1024] psum tile in ~512 cycles, and a
  single N=1024 exp call covers the pair.
- Units are (query-block of 512, pair); sp is double-buffered so the PE
  writes scores for chunk j+1 while ACT exps chunk j; attn@v (with the
  ones-column denominator trick) drains at bf16 from SBUF behind exp.
- All projection / output-projection work is interleaved into the PE's
  slack inside the jc loops (useful filler instead of the old HAM-bridge
  dummies), and per-chunk kT/qT/vv tiles let the first exp start ~8us in.

All matmuls in float32r (1 cycle/row). Softmax skips max-subtraction
(|s| < ~55 whp, exp fits fp32/bf16) exactly like the reference within tol.
"""
import sys

sys.path.insert(0, "/opt/trn_rl_repo")
import numpy as np

B, S, D, H, HD = 4, 2048, 512, 8, 64
HPC = 4          # heads per core
DQ = HPC * HD    # 256 projection dims per core
NCORES = 8
VW = HD + 1      # v block width incl. ones column (65)
QB = 512         # query block
NQB = S // QB    # 4
NJC = S // 128   # 16 key chunks

_cache = {}


def _build_nc():
    import concourse.bacc as bacc
    import concourse.mybir as mybir
    import concourse.tile as tile

    F32, F32R = mybir.dt.float32, mybir.dt.float32r
    BF16 = mybir.dt.bfloat16
    EXP = mybir.ActivationFunctionType.Exp

    nc = bacc.Bacc("TRN2", target_bir_lowering=False, debug=False)

    # xTc[sc] = x[b].T[:, sc*512:(sc+1)*512], contiguous per chunk so the
    # input DMAs read large sequential DRAM blocks (single FIFO HW queue).
    xTc = nc.dram_tensor("xTc", [4, D, 512], F32R, kind="ExternalInput")
    wqT = nc.dram_tensor("wqT", [D, DQ], F32R, kind="ExternalInput")
    wkT = nc.dram_tensor("wkT", [D, DQ], F32R, kind="ExternalInput")
    wvT = nc.dram_tensor("wvT", [D, DQ], F32R, kind="ExternalInput")
    woT = nc.dram_tensor("woT", [DQ, D], F32R, kind="ExternalInput")
    # outTc[m, qb] = outT[m*128:(m+1)*128, qb*512:(qb+1)*512] (host unpacks)
    outTc = nc.dram_tensor("outTc", [4, NQB, 128, 512], F32,
                           kind="ExternalOutput")
    NU = 2 * NQB * 2  # (pair, qb, head) rows
    scr_sums = nc.dram_tensor("scr_sums", [NU, QB], F32)
    scr_recip = nc.dram_tensor("scr_recip", [NU, QB], F32)

    with tile.TileContext(nc) as tc:
        with tc.tile_pool(name="sb", bufs=1) as sb:
            psum = tc.tile_pool(name="psum", bufs=1, space="PSUM")
            pp = psum.__enter__()

            # ---- input DMAs, priority order for earliest first-exp:
            # wk/wq pair-0 column slices -> x chunk 0 -> wv -> x chunk 1 ->
            # pair-1 weight slices -> x chunks 2,3 -> wo.  (Single HW DMA
            # queue is FIFO, so emission order = landing order.)
            wk = [sb.tile([128, DQ], F32R, tag=f"wk{d}", name=f"wk{d}")
                  for d in range(4)]
            wq = [sb.tile([128, DQ], F32R, tag=f"wq{d}", name=f"wq{d}")
                  for d in range(4)]
            xt = [sb.tile([128, S], F32R, tag=f"xt{d}", name=f"xt{d}")
                  for d in range(4)]
            for lst, dram in ((wk, wkT), (wq, wqT)):
                for d in range(4):
                    nc.sync.dma_start(out=lst[d][:, 0:128],
                                      in_=dram[128 * d:128 * (d + 1), 0:128])
            for d in range(4):
                nc.sync.dma_start(out=xt[d][:, 0:512],
                                  in_=xTc[0, 128 * d:128 * (d + 1), :])
            wv = []
            for d in range(4):
                t = sb.tile([128, DQ], F32R, tag=f"wv{d}", name=f"wv{d}")
                nc.sync.dma_start(out=t[:], in_=wvT[128 * d:128 * (d + 1), :])
                wv.append(t)
            for d in range(4):
                nc.sync.dma_start(out=xt[d][:, 512:1024],
                                  in_=xTc[1, 128 * d:128 * (d + 1), :])
            for lst, dram in ((wk, wkT), (wq, wqT)):
                for d in range(4):
                    nc.sync.dma_start(out=lst[d][:, 128:256],
                                      in_=dram[128 * d:128 * (d + 1), 128:256])
            for sc in range(2, 4):
                for d in range(4):
                    nc.sync.dma_start(
                        out=xt[d][:, sc * 512:(sc + 1) * 512],
                        in_=xTc[sc, 128 * d:128 * (d + 1), :])
            wo = []
            for kc in range(4):
                t = sb.tile([64, D], F32R, tag=f"wo{kc}", name=f"wo{kc}")
                nc.sync.dma_start(out=t[:], in_=woT[64 * kc:64 * (kc + 1), :])
                wo.append(t)

            # ---- ACT table pre-load: tiny exp at t0 (hidden under DMA) ----
            dumm = sb.tile([128, 8], F32, tag="dumm", name="dumm")
            nc.vector.memset(dumm[:], 0.0)
            dumo = sb.tile([128, 8], F32, tag="dumo", name="dumo")
            nc.scalar.activation(dumo[:], dumm[:], EXP)

            # ---- persistent tiles ----
            # kT/qT per (pair, chunk): partitions 0-63 = head 2p, 64-127 = 2p+1
            kTt = [[sb.tile([128, 512], F32R, tag=f"kT{p}_{sc}", name=f"kT{p}_{sc}")
                    for sc in range(4)] for p in range(2)]
            qTt = [[sb.tile([128, 512], F32R, tag=f"qT{p}_{qb}", name=f"qT{p}_{qb}")
                    for qb in range(NQB)] for p in range(2)]
            vvt = [sb.tile([128, HPC * VW], BF16, tag=f"vv{jc}", name=f"vv{jc}")
                   for jc in range(NJC)]
            oTn = [[sb.tile([64, QB], F32R, tag=f"oTn{h}_{qb}", name=f"oTn{h}_{qb}")
                    for qb in range(NQB)] for h in range(HPC)]
            ones32 = sb.tile([128, 1], F32, tag="ones32", name="ones32")
            nc.vector.memset(ones32[:], 1.0)

            # ---- proj group emitters (each ~1-2k PE cycles + a DVE copy) ----
            def kq_group(dst, wsb, p, col0):
                # dst[:, :] = (w pair-slice).T @ x[:, col0:col0+512]
                ps = pp.tile([128, 512], F32, tag="pg", bufs=2, name="ps")
                for d in range(4):
                    nc.tensor.matmul(
                        ps[:], wsb[d][:, p * 128:(p + 1) * 128],
                        xt[d][:, col0:col0 + 512],
                        start=(d == 0), stop=(d == 3))
                nc.vector.tensor_copy(out=dst[:], in_=ps[:])

            def vv_group(jc):
                psv = pp.tile([128, 512], F32, tag="pg", bufs=2, name="psv")
                for d in range(4):
                    nc.tensor.matmul(
                        psv[:, 0:DQ], xt[d][:, jc * 128:(jc + 1) * 128],
                        wv[d][:, :], start=(d == 0), stop=(d == 3))
                vv_ones = vvt[jc][:, :].rearrange(
                    "p (g w) -> p g w", w=VW)[:, :, HD:HD + 1]
                nc.vector.tensor_copy(
                    out=vv_ones, in_=ones32[:].to_broadcast((128, HPC, 1)))
                for h in range(HPC):
                    nc.vector.tensor_copy(
                        out=vvt[jc][:, VW * h: VW * h + HD],
                        in_=psv[:, HD * h: HD * (h + 1)])

            def outproj_group(m, qb):
                # outT[m-chunk, qb-block] = sum_h woT[h-dims, m-chunk].T @ oTn
                po = pp.tile([128, 512], F32, tag="pg", bufs=2, name="po")
                for kc in range(4):
                    nc.tensor.matmul(
                        po[:], wo[kc][:, m * 128:(m + 1) * 128],
                        oTn[kc][qb][:], start=(kc == 0), stop=(kc == 3))
                ob = sb.tile([128, 512], F32, tag="ob", bufs=4, name="ob")
                nc.vector.tensor_copy(out=ob[:], in_=po[:])
                nc.sync.dma_start(out=outTc[m, qb], in_=ob[:])

            # ---- explicit filler schedule: unit u = qb*2 + p; slots[u][jc]
            # is a list of closures emitted right after attnv(jc-1), i.e. in
            # the PE's ACT-bound slack.  Placement respects (a) emission
            # before consumption, (b) input-DMA landing times (so a stalled
            # filler never blocks the FIFO PE queue ahead of scores).
            slots = [dict() for _ in range(2 * NQB)]

            def put(u, jc, fn):
                slots[u].setdefault(jc, []).append(fn)

            # unit 0 = (qb0, p0): stream in remaining vv + kT chunks
            for k in range(2, NJC):
                put(0, k - 2, lambda jc=k: vv_group(jc))
            put(0, 3, lambda: kq_group(kTt[0][1], wk, 0, 512))
            put(0, 5, lambda: kq_group(kTt[0][2], wk, 0, 1024))
            put(0, 9, lambda: kq_group(kTt[0][3], wk, 0, 1536))
            put(0, 11, lambda: kq_group(kTt[1][0], wk, 1, 0))
            put(0, 13, lambda: kq_group(kTt[1][1], wk, 1, 512))
            put(0, 15, lambda: kq_group(qTt[1][0], wq, 1, 0))
            # unit 1 = (qb0, p1): its own late kT chunks + next qT
            put(1, 1, lambda: kq_group(kTt[1][2], wk, 1, 1024))
            put(1, 3, lambda: kq_group(kTt[1][3], wk, 1, 1536))
            put(1, 9, lambda: kq_group(qTt[0][1], wq, 0, 512))
            for qb in range(1, NQB):
                u0 = 2 * qb
                # outproj of the previous qblock goes in the ODD unit (a
                # full unit, ~18us, after the epilogues that produce its
                # oTn inputs -- their DMA round-trip chain must not stall
                # the PE FIFO in front of scores).
                for m, j in enumerate((5, 7, 11, 13)):
                    put(u0 + 1, j, lambda m=m, qb=qb: outproj_group(m, qb - 1))
                put(u0, 9, lambda qb=qb: kq_group(qTt[1][qb], wq, 1, qb * 512))
                if qb + 1 < NQB:
                    put(u0 + 1, 9, lambda qb=qb: kq_group(
                        qTt[0][qb + 1], wq, 0, (qb + 1) * 512))

            # ---- phase P: minimal prefix before unit (qb0, p0) ----
            with nc.named_scope("proj"):
                kq_group(kTt[0][0], wk, 0, 0)
                kq_group(qTt[0][0], wq, 0, 0)
                vv_group(0)
                vv_group(1)

            # ---- attention units ----
            def epilogue(p, qb, e, op_t):
                h = 2 * p + e
                r = (p * NQB + qb) * 2 + e
                otu = sb.tile([65, QB], F32, tag="otu", bufs=2, name="otu")
                nc.vector.tensor_copy(out=otu[:], in_=op_t[:])
                nc.sync.dma_start(out=scr_sums[r:r + 1, :], in_=otu[64:65, :])
                sumsT = sb.tile([128, 4], F32, tag="sumsT", bufs=2, name="sumsT")
                nc.sync.dma_start(
                    out=sumsT[:],
                    in_=scr_sums[r:r + 1, :].rearrange("o (c p) -> (o p) c", p=128))
                recipT = sb.tile([128, 4], F32, tag="recipT", bufs=2, name="recipT")
                nc.vector.reciprocal(recipT[:], sumsT[:])
                nc.sync.dma_start(
                    out=scr_recip[r:r + 1, :].rearrange("o (c p) -> (o p) c", p=128),
                    in_=recipT[:])
                rb = sb.tile([64, QB], F32, tag="rb", bufs=2, name="rb")
                nc.sync.dma_start(
                    out=rb[:], in_=scr_recip[r:r + 1, :].to_broadcast((64, QB)))
                nc.vector.tensor_mul(
                    out=oTn[h][qb][:], in0=otu[0:64, :], in1=rb[:])

            with nc.named_scope("attn"):
                for u in range(2 * NQB):
                    qb, p = u // 2, u % 2
                    op0 = pp.tile([65, QB], F32, tag="op0", bufs=1, name="op0")
                    op1 = pp.tile([65, QB], F32, tag="op1", bufs=1, name="op1")
                    ats = [None] * 16
                    # software-pipelined: attnv lags scores/exp by one jc so
                    # the FIFO PE queue never parks scores behind an
                    # exp-dependent matmul.
                    for jc in range(16):
                        sp = pp.tile([128, 1024], F32, tag="sp", bufs=2,
                                     name="sp")
                        sc, c0 = jc // 4, (jc % 4) * 128
                        for e in range(2):
                            nc.tensor.matmul(
                                sp[:, e * 512:(e + 1) * 512],
                                kTt[p][sc][64 * e:64 * e + 64, c0:c0 + 128],
                                qTt[p][qb][64 * e:64 * e + 64, :],
                                start=True, stop=True,
                                tile_position=(64 * e, 0))
                        at = sb.tile([128, 1024], BF16, tag="at", bufs=3,
                                     name="at")
                        nc.scalar.activation(at[:], sp[:], EXP)
                        ats[jc] = at

                        def attnv(j):
                            for e, op_t in ((0, op0), (1, op1)):
                                nc.tensor.matmul(
                                    op_t[:],
                                    vvt[j][:, VW * (2 * p + e):
                                           VW * (2 * p + e) + VW],
                                    ats[j][:, e * 512:(e + 1) * 512],
                                    start=(j == 0), stop=(j == 15))
                        if jc > 0:
                            attnv(jc - 1)
                        for fn in slots[u].get(jc, ()):
                            fn()
                    attnv(15)
                    for e, op_t in ((0, op0), (1, op1)):
                        epilogue(p, qb, e, op_t)

            with nc.named_scope("outproj"):
                for m in range(4):
                    outproj_group(m, NQB - 1)

            psum.__exit__(None, None, None)

    nc.compile()
    return nc


def _get_nc():
    if "nc" not in _cache:
        _cache["nc"] = _build_nc()
    return _cache["nc"]


def _in_maps(x, w_qkv, w_out):
    x = np.asarray(x, dtype=np.float32)
    w_qkv = np.asarray(w_qkv, dtype=np.float32)
    w_out = np.asarray(w_out, dtype=np.float32)
    maps = []
    for c in range(NCORES):
        b, qh = c // 2, c % 2
        r0 = qh * DQ
        xT = x[b].T  # [D, S]
        xTc = np.ascontiguousarray(
            xT.reshape(D, 4, 512).transpose(1, 0, 2))  # [4, D, 512]
        maps.append({
            "xTc": xTc,
            "wqT": np.ascontiguousarray(w_qkv[r0:r0 + DQ].T),
            "wkT": np.ascontiguousarray(w_qkv[D + r0:D + r0 + DQ].T),
            "wvT": np.ascontiguousarray(w_qkv[2 * D + r0:2 * D + r0 + DQ].T),
            "woT": np.ascontiguousarray(w_out[:, r0:r0 + DQ].T),
        })
    return maps


def _gather(results):
    out = np.empty((B, S, D), np.float32)
    for b in range(B):
        acc = results[2 * b]["outTc"] + results[2 * b + 1]["outTc"]
        # [4(m), NQB, 128, 512] -> outT [D, S] -> out [S, D]
        outT = acc.transpose(0, 2, 1, 3).reshape(D, S)
        out[b] = outT.T
    return out


def run(x, w_qkv, w_out, trace=False):
    from concourse.bass_utils import run_bass_kernel_spmd

    nc = _get_nc()
    res = run_bass_kernel_spmd(
        nc, _in_maps(x, w_qkv, w_out), core_ids=list(range(NCORES)), trace=trace,
    )
    return _gather(res.results), res


def kernel(x, w_qkv, w_out):
    out, _ = run(x, w_qkv, w_out)
    return out


# revision 14
# speedup vs baseline: 1.3682x; 1.0509x over previous
"""Multi-head attention (B=4, S=2048, D=512, H=8) on 8 trn2 cores.

Sharding: core c handles batch b=c//2 and the head-quad qh=c%2 (heads
4*qh..4*qh+3, i.e. 2 head-PAIRS). The kernel is restructured around keeping
the Scalar (ACT) engine -- which does the softmax exp and is the true
bottleneck at 1 elem/cycle/lane @1.2GHz (~147us/core) -- saturated:

- Scores for a head PAIR run concurrently on the PE via row tiling
  (tile_position (0,0)/(64,0), K=64 each): both heads' scores for one
  128-key chunk land in one [128, 1024] psum tile in ~512 cycles, and a
  single N=1024 exp call covers the pair.
- Units are (query-block of 512, pair); sp is double-buffered so the PE
  writes scores for chunk j+1 while ACT exps chunk j; attn@v (with the
  ones-column denominator trick) drains at bf16 from SBUF behind exp.
- All projection / output-projection work is interleaved into the PE's
  slack inside the jc loops (useful filler instead of the old HAM-bridge
  dummies), and per-chunk kT/qT/vv tiles let the first exp start ~8us in.

All matmuls in float32r (1 cycle/row). Softmax skips max-subtraction
(|s| < ~55 whp, exp fits fp32/bf16) exactly like the reference within tol.
"""
import sys

sys.path.insert(0, "/opt/trn_rl_repo")
import numpy as np

B, S, D, H, HD = 4, 2048, 512, 8, 64
HPC = 4          # heads per core
DQ = HPC * HD    # 256 projection dims per core
NCORES = 8
VW = HD + 1      # v block width incl. ones column (65)
QB = 512         # query block
NQB = S // QB    # 4
NJC = S // 128   # 16 key chunks

_cache = {}


def _build_nc():
    import concourse.bacc as bacc
    import concourse.mybir as mybir
    import concourse.tile as tile

    F32, F32R = mybir.dt.float32, mybir.dt.float32r
    BF16 = mybir.dt.bfloat16
    EXP = mybir.ActivationFunctionType.Exp

    nc = bacc.Bacc("TRN2", target_bir_lowering=False, debug=False)

    # xTc[sc] = x[b].T[:, sc*512:(sc+1)*512], contiguous per chunk so the
    # input DMAs read large sequential DRAM blocks (single FIFO HW queue).
    xTc = nc.dram_tensor("xTc", [4, D, 512], F32R, kind="ExternalInput")
    wqT = nc.dram_tensor("wqT", [D, DQ], F32R, kind="ExternalInput")
    wkT = nc.dram_tensor("wkT", [D, DQ], F32R, kind="ExternalInput")
    wvT = nc.dram_tensor("wvT", [D, DQ], F32R, kind="ExternalInput")
    woT = nc.dram_tensor("woT", [DQ, D], F32R, kind="ExternalInput")
    # outTc[m, qb] = outT[m*128:(m+1)*128, qb*512:(qb+1)*512] (host unpacks)
    outTc = nc.dram_tensor("outTc", [4, NQB, 128, 512], F32,
                           kind="ExternalOutput")
    NU = 2 * NQB * 2  # (pair, qb, head) rows
    scr_sums = nc.dram_tensor("scr_sums", [NU, QB], F32)
    scr_recip = nc.dram_tensor("scr_recip", [NU, QB], F32)

    with tile.TileContext(nc) as tc:
        with tc.tile_pool(name="sb", bufs=1) as sb:
            psum = tc.tile_pool(name="psum", bufs=1, space="PSUM")
            pp = psum.__enter__()

            # ---- input DMAs, priority order for earliest first-exp:
            # wk/wq pair-0 column slices -> x chunk 0 -> wv -> x chunk 1 ->
            # pair-1 weight slices -> x chunks 2,3 -> wo.  (Single HW DMA
            # queue is FIFO, so emission order = landing order.)
            wk = [sb.tile([128, DQ], F32R, tag=f"wk{d}", name=f"wk{d}")
                  for d in range(4)]
            wq = [sb.tile([128, DQ], F32R, tag=f"wq{d}", name=f"wq{d}")
                  for d in range(4)]
            xt = [sb.tile([128, S], F32R, tag=f"xt{d}", name=f"xt{d}")
                  for d in range(4)]
            for lst, dram in ((wk, wkT), (wq, wqT)):
                for d in range(4):
                    nc.sync.dma_start(out=lst[d][:, 0:128],
                                      in_=dram[128 * d:128 * (d + 1), 0:128])
            for d in range(4):
                nc.sync.dma_start(out=xt[d][:, 0:512],
                                  in_=xTc[0, 128 * d:128 * (d + 1), :])
            wv = []
            for d in range(4):
                t = sb.tile([128, DQ], F32R, tag=f"wv{d}", name=f"wv{d}")
                nc.sync.dma_start(out=t[:], in_=wvT[128 * d:128 * (d + 1), :])
                wv.append(t)
            for d in range(4):
                nc.sync.dma_start(out=xt[d][:, 512:1024],
                                  in_=xTc[1, 128 * d:128 * (d + 1), :])
            for lst, dram in ((wk, wkT), (wq, wqT)):
                for d in range(4):
                    nc.sync.dma_start(out=lst[d][:, 128:256],
                                      in_=dram[128 * d:128 * (d + 1), 128:256])
            for sc in range(2, 4):
                for d in range(4):
                    nc.sync.dma_start(
                        out=xt[d][:, sc * 512:(sc + 1) * 512],
                        in_=xTc[sc, 128 * d:128 * (d + 1), :])
            wo = []
            for kc in range(4):
                t = sb.tile([64, D], F32R, tag=f"wo{kc}", name=f"wo{kc}")
                nc.sync.dma_start(out=t[:], in_=woT[64 * kc:64 * (kc + 1), :])
                wo.append(t)

            # ---- ACT table pre-load: tiny exp at t0 (hidden under DMA) ----
            dumm = sb.tile([128, 8], F32, tag="dumm", name="dumm")
            nc.vector.memset(dumm[:], 0.0)
            dumo = sb.tile([128, 8], F32, tag="dumo", name="dumo")
            nc.scalar.activation(dumo[:], dumm[:], EXP)

            # ---- persistent tiles ----
            # kT/qT per (pair, chunk): partitions 0-63 = head 2p, 64-127 = 2p+1
            kTt = [[sb.tile([128, 512], F32R, tag=f"kT{p}_{sc}", name=f"kT{p}_{sc}")
                    for sc in range(4)] for p in range(2)]
            qTt = [[sb.tile([128, 512], F32R, tag=f"qT{p}_{qb}", name=f"qT{p}_{qb}")
                    for qb in range(NQB)] for p in range(2)]
            vvt = [sb.tile([128, HPC * VW], BF16, tag=f"vv{jc}", name=f"vv{jc}")
                   for jc in range(NJC)]
            oTn = [[sb.tile([64, QB], F32R, tag=f"oTn{h}_{qb}", name=f"oTn{h}_{qb}")
                    for qb in range(NQB)] for h in range(HPC)]
            ones32 = sb.tile([128, 1], F32, tag="ones32", name="ones32")
            nc.vector.memset(ones32[:], 1.0)

            # ---- proj group emitters (each ~1-2k PE cycles + a DVE copy) ----
            def kq_group(dst, wsb, p, col0):
                # dst[:, :] = (w pair-slice).T @ x[:, col0:col0+512]
                ps = pp.tile([128, 512], F32, tag="pg", bufs=2, name="ps")
                for d in range(4):
                    nc.tensor.matmul(
                        ps[:], wsb[d][:, p * 128:(p + 1) * 128],
                        xt[d][:, col0:col0 + 512],
                        start=(d == 0), stop=(d == 3))
                nc.vector.tensor_copy(out=dst[:], in_=ps[:])

            def vv_group(jc):
                psv = pp.tile([128, 512], F32, tag="pg", bufs=2, name="psv")
                for d in range(4):
                    nc.tensor.matmul(
                        psv[:, 0:DQ], xt[d][:, jc * 128:(jc + 1) * 128],
                        wv[d][:, :], start=(d == 0), stop=(d == 3))
                vv_ones = vvt[jc][:, :].rearrange(
                    "p (g w) -> p g w", w=VW)[:, :, HD:HD + 1]
                nc.vector.tensor_copy(
                    out=vv_ones, in_=ones32[:].to_broadcast((128, HPC, 1)))
                for h in range(HPC):
                    nc.vector.tensor_copy(
                        out=vvt[jc][:, VW * h: VW * h + HD],
                        in_=psv[:, HD * h: HD * (h + 1)])

            def outproj_group(m, qb):
                # outT[m-chunk, qb-block] = sum_h woT[h-dims, m-chunk].T @ oTn
                po = pp.tile([128, 512], F32, tag="pg", bufs=2, name="po")
                for kc in range(4):
                    nc.tensor.matmul(
                        po[:], wo[kc][:, m * 128:(m + 1) * 128],
                        oTn[kc][qb][:], start=(kc == 0), stop=(kc == 3))
                ob = sb.tile([128, 512], F32, tag="ob", bufs=4, name="ob")
                nc.vector.tensor_copy(out=ob[:], in_=po[:])
                nc.sync.dma_start(out=outTc[m, qb], in_=ob[:])

            # ---- explicit filler schedule: unit u = qb*2 + p; slots[u][jc]
            # is a list of closures emitted right after attnv(jc-1), i.e. in
            # the PE's ACT-bound slack.  Placement respects (a) emission
            # before consumption, (b) input-DMA landing times (so a stalled
            # filler never blocks the FIFO PE queue ahead of scores).
            slots = [dict() for _ in range(2 * NQB)]

            def put(u, jc, fn):
                slots[u].setdefault(jc, []).append(fn)

            # unit 0 = (qb0, p0): stream in remaining vv + kT chunks
            for k in range(2, NJC):
                put(0, k - 2, lambda jc=k: vv_group(jc))
            put(0, 3, lambda: kq_group(kTt[0][1], wk, 0, 512))
            put(0, 5, lambda: kq_group(kTt[0][2], wk, 0, 1024))
            put(0, 9, lambda: kq_group(kTt[0][3], wk, 0, 1536))
            put(0, 11, lambda: kq_group(kTt[1][0], wk, 1, 0))
            put(0, 13, lambda: kq_group(kTt[1][1], wk, 1, 512))
            put(0, 15, lambda: kq_group(qTt[1][0], wq, 1, 0))
            # unit 1 = (qb0, p1): its own late kT chunks + next qT
            put(1, 1, lambda: kq_group(kTt[1][2], wk, 1, 1024))
            put(1, 3, lambda: kq_group(kTt[1][3], wk, 1, 1536))
            put(1, 9, lambda: kq_group(qTt[0][1], wq, 0, 512))
            for qb in range(1, NQB):
                u0 = 2 * qb
                # outproj of the previous qblock goes in the ODD unit (a
                # full unit, ~18us, after the epilogues that produce its
                # oTn inputs -- their DMA round-trip chain must not stall
                # the PE FIFO in front of scores).
                for m, j in enumerate((5, 7, 11, 13)):
                    put(u0 + 1, j, lambda m=m, qb=qb: outproj_group(m, qb - 1))
                put(u0, 9, lambda qb=qb: kq_group(qTt[1][qb], wq, 1, qb * 512))
                if qb + 1 < NQB:
                    put(u0 + 1, 9, lambda qb=qb: kq_group(
                        qTt[0][qb + 1], wq, 0, (qb + 1) * 512))

            # ---- phase P: minimal prefix before unit (qb0, p0) ----
            with nc.named_scope("proj"):
                kq_group(kTt[0][0], wk, 0, 0)
                kq_group(qTt[0][0], wq, 0, 0)
                vv_group(0)
                vv_group(1)

            # ---- attention units ----
            def epilogue(p, qb, e, op_t):
                h = 2 * p + e
                r = (p * NQB + qb) * 2 + e
                otu = sb.tile([65, QB], F32, tag="otu", bufs=2, name="otu")
                nc.vector.tensor_copy(out=otu[:], in_=op_t[:])
                # DMA chain rides the (otherwise idle) GpSimd SWDGE queue so
                # its sem-waits never head-of-line-block the Sync queue that
                # carries the input/output streams.
                nc.gpsimd.dma_start(out=scr_sums[r:r + 1, :], in_=otu[64:65, :])
                sumsT = sb.tile([128, 4], F32, tag="sumsT", bufs=2, name="sumsT")
                nc.gpsimd.dma_start(
                    out=sumsT[:],
                    in_=scr_sums[r:r + 1, :].rearrange("o (c p) -> (o p) c", p=128))
                recipT = sb.tile([128, 4], F32, tag="recipT", bufs=2, name="recipT")
                nc.vector.reciprocal(recipT[:], sumsT[:])
                nc.gpsimd.dma_start(
                    out=scr_recip[r:r + 1, :].rearrange("o (c p) -> (o p) c", p=128),
                    in_=recipT[:])
                rb = sb.tile([64, QB], F32, tag="rb", bufs=2, name="rb")
                nc.gpsimd.dma_start(
                    out=rb[:], in_=scr_recip[r:r + 1, :].to_broadcast((64, QB)))
                nc.vector.tensor_mul(
                    out=oTn[h][qb][:], in0=otu[0:64, :], in1=rb[:])

            with nc.named_scope("attn"):
                for u in range(2 * NQB):
                    qb, p = u // 2, u % 2
                    op0 = pp.tile([65, QB], F32, tag="op0", bufs=1, name="op0")
                    op1 = pp.tile([65, QB], F32, tag="op1", bufs=1, name="op1")
                    ats = [None] * 16
                    # software-pipelined: attnv lags scores/exp by one jc so
                    # the FIFO PE queue never parks scores behind an
                    # exp-dependent matmul.
                    for jc in range(16):
                        sp = pp.tile([128, 1024], F32, tag="sp", bufs=2,
                                     name="sp")
                        sc, c0 = jc // 4, (jc % 4) * 128
                        for e in range(2):
                            nc.tensor.matmul(
                                sp[:, e * 512:(e + 1) * 512],
                                kTt[p][sc][64 * e:64 * e + 64, c0:c0 + 128],
                                qTt[p][qb][64 * e:64 * e + 64, :],
                                start=True, stop=True,
                                tile_position=(64 * e, 0))
                        at = sb.tile([128, 1024], BF16, tag="at", bufs=3,
                                     name="at")
                        nc.scalar.activation(at[:], sp[:], EXP)
                        ats[jc] = at

                        def attnv(j):
                            for e, op_t in ((0, op0), (1, op1)):
                                nc.tensor.matmul(
                                    op_t[:],
                                    vvt[j][:, VW * (2 * p + e):
                                           VW * (2 * p + e) + VW],
                                    ats[j][:, e * 512:(e + 1) * 512],
                                    start=(j == 0), stop=(j == 15))
                        if jc > 0:
                            attnv(jc - 1)
                        for fn in slots[u].get(jc, ()):
                            fn()
                    attnv(15)
                    for e, op_t in ((0, op0), (1, op1)):
                        epilogue(p, qb, e, op_t)

            with nc.named_scope("outproj"):
                for m in range(4):
                    outproj_group(m, NQB - 1)

            psum.__exit__(None, None, None)

    nc.compile()
    return nc


def _get_nc():
    if "nc" not in _cache:
        _cache["nc"] = _build_nc()
    return _cache["nc"]


def _in_maps(x, w_qkv, w_out):
    x = np.asarray(x, dtype=np.float32)
    w_qkv = np.asarray(w_qkv, dtype=np.float32)
    w_out = np.asarray(w_out, dtype=np.float32)
    maps = []
    for c in range(NCORES):
        b, qh = c // 2, c % 2
        r0 = qh * DQ
        xT = x[b].T  # [D, S]
        xTc = np.ascontiguousarray(
            xT.reshape(D, 4, 512).transpose(1, 0, 2))  # [4, D, 512]
        maps.append({
            "xTc": xTc,
            "wqT": np.ascontiguousarray(w_qkv[r0:r0 + DQ].T),
            "wkT": np.ascontiguousarray(w_qkv[D + r0:D + r0 + DQ].T),
            "wvT": np.ascontiguousarray(w_qkv[2 * D + r0:2 * D + r0 + DQ].T),
            "woT": np.ascontiguousarray(w_out[:, r0:r0 + DQ].T),
        })
    return maps


def _gather(results):
    out = np.empty((B, S, D), np.float32)
    for b in range(B):
        acc = results[2 * b]["outTc"] + results[2 * b + 1]["outTc"]
        # [4(m), NQB, 128, 512] -> outT [D, S] -> out [S, D]
        outT = acc.transpose(0, 2, 1, 3).reshape(D, S)
        out[b] = outT.T
    return out


def run(x, w_qkv, w_out, trace=False):
    from concourse.bass_utils import run_bass_kernel_spmd

    nc = _get_nc()
    res = run_bass_kernel_spmd(
        nc, _in_maps(x, w_qkv, w_out), core_ids=list(range(NCORES)), trace=trace,
    )
    return _gather(res.results), res


def kernel(x, w_qkv, w_out):
    out, _ = run(x, w_qkv, w_out)
    return out


# revision 20
# speedup vs baseline: 1.6588x; 1.2125x over previous
"""Multi-head attention (B=4, S=2048, D=512, H=8) on 8 trn2 cores.

Sharding: core c handles batch b=c//2 and the head-quad qh=c%2 (heads
4*qh..4*qh+3, i.e. 2 head-PAIRS). The kernel is restructured around keeping
the Scalar (ACT) engine -- which does the softmax exp and is the true
bottleneck at 1 elem/cycle/lane @1.2GHz (~147us/core) -- saturated:

- Scores for a head PAIR run concurrently on the PE via row tiling
  (tile_position (0,0)/(64,0), K=64 each): both heads' scores for one
  128-key chunk land in one [128, 1024] psum tile in ~512 cycles, and a
  single N=1024 exp call covers the pair.
- Units are (query-block of 512, pair); sp is double-buffered so the PE
  writes scores for chunk j+1 while ACT exps chunk j; attn@v (with the
  ones-column denominator trick) drains at bf16 from SBUF behind exp.
- All projection / output-projection work is interleaved into the PE's
  slack inside the jc loops (useful filler instead of the old HAM-bridge
  dummies), and per-chunk kT/qT/vv tiles let the first exp start ~8us in.

All matmuls in float32r (1 cycle/row). Softmax skips max-subtraction
(|s| < ~55 whp, exp fits fp32/bf16) exactly like the reference within tol.
"""
import sys

sys.path.insert(0, "/opt/trn_rl_repo")
import numpy as np

B, S, D, H, HD = 4, 2048, 512, 8, 64
HPC = 4          # heads per core
DQ = HPC * HD    # 256 projection dims per core
NCORES = 8
VW = HD + 1      # v block width incl. ones column (65)
QB = 512         # query block
NQB = S // QB    # 4
NJC = S // 128   # 16 key chunks

_cache = {}


def _build_nc():
    import concourse.bacc as bacc
    import concourse.mybir as mybir
    import concourse.tile as tile

    F32, F32R = mybir.dt.float32, mybir.dt.float32r
    BF16 = mybir.dt.bfloat16
    EXP = mybir.ActivationFunctionType.Exp

    nc = bacc.Bacc("TRN2", target_bir_lowering=False, debug=False)

    # xTc[sc] = x[b].T[:, sc*512:(sc+1)*512], contiguous per chunk so the
    # input DMAs read large sequential DRAM blocks (single FIFO HW queue).
    xTc = nc.dram_tensor("xTc", [4, D, 512], F32R, kind="ExternalInput")
    wqT = nc.dram_tensor("wqT", [D, DQ], F32R, kind="ExternalInput")
    wkT = nc.dram_tensor("wkT", [D, DQ], F32R, kind="ExternalInput")
    wvT = nc.dram_tensor("wvT", [D, DQ], F32R, kind="ExternalInput")
    woT = nc.dram_tensor("woT", [DQ, D], F32R, kind="ExternalInput")
    # outTc[m, qb] = outT[m*128:(m+1)*128, qb*512:(qb+1)*512] (host unpacks)
    outTc = nc.dram_tensor("outTc", [4, NQB, 128, 512], F32,
                           kind="ExternalOutput")
    identT = nc.dram_tensor("identT", [128, 128], F32, kind="ExternalInput")

    with tile.TileContext(nc) as tc:
        with tc.tile_pool(name="sb", bufs=1) as sb:
            psum = tc.tile_pool(name="psum", bufs=1, space="PSUM")
            pp = psum.__enter__()

            # ---- input DMAs, priority order for earliest first-exp:
            # wk/wq pair-0 column slices -> x chunk 0 -> wv -> x chunk 1 ->
            # pair-1 weight slices -> x chunks 2,3 -> wo.  (Single HW DMA
            # queue is FIFO, so emission order = landing order.)
            wk = [sb.tile([128, DQ], F32R, tag=f"wk{d}", name=f"wk{d}")
                  for d in range(4)]
            wq = [sb.tile([128, DQ], F32R, tag=f"wq{d}", name=f"wq{d}")
                  for d in range(4)]
            xt = [sb.tile([128, S], F32R, tag=f"xt{d}", name=f"xt{d}")
                  for d in range(4)]
            for lst, dram in ((wk, wkT), (wq, wqT)):
                for d in range(4):
                    nc.sync.dma_start(out=lst[d][:, 0:128],
                                      in_=dram[128 * d:128 * (d + 1), 0:128])
            for d in range(4):
                nc.sync.dma_start(out=xt[d][:, 0:512],
                                  in_=xTc[0, 128 * d:128 * (d + 1), :])
            wv = []
            for d in range(4):
                t = sb.tile([128, DQ], F32R, tag=f"wv{d}", name=f"wv{d}")
                nc.sync.dma_start(out=t[:], in_=wvT[128 * d:128 * (d + 1), :])
                wv.append(t)
            for d in range(4):
                nc.sync.dma_start(out=xt[d][:, 512:1024],
                                  in_=xTc[1, 128 * d:128 * (d + 1), :])
            for lst, dram in ((wk, wkT), (wq, wqT)):
                for d in range(4):
                    nc.sync.dma_start(out=lst[d][:, 128:256],
                                      in_=dram[128 * d:128 * (d + 1), 128:256])
            for sc in range(2, 4):
                for d in range(4):
                    nc.sync.dma_start(
                        out=xt[d][:, sc * 512:(sc + 1) * 512],
                        in_=xTc[sc, 128 * d:128 * (d + 1), :])
            wo = []
            for kc in range(4):
                t = sb.tile([64, D], F32R, tag=f"wo{kc}", name=f"wo{kc}")
                nc.sync.dma_start(out=t[:], in_=woT[64 * kc:64 * (kc + 1), :])
                wo.append(t)
            ident = sb.tile([128, 128], F32, tag="ident", name="ident")
            nc.sync.dma_start(out=ident[:], in_=identT[:, :])

            # ---- ACT table pre-load: tiny exp at t0 (hidden under DMA) ----
            dumm = sb.tile([128, 8], F32, tag="dumm", name="dumm")
            nc.vector.memset(dumm[:], 0.0)
            dumo = sb.tile([128, 8], F32, tag="dumo", name="dumo")
            nc.scalar.activation(dumo[:], dumm[:], EXP)

            # ---- persistent tiles ----
            # kT/qT per (pair, chunk): partitions 0-63 = head 2p, 64-127 = 2p+1
            kTt = [[sb.tile([128, 512], F32R, tag=f"kT{p}_{sc}", name=f"kT{p}_{sc}")
                    for sc in range(4)] for p in range(2)]
            qTt = [[sb.tile([128, 512], F32R, tag=f"qT{p}_{qb}", name=f"qT{p}_{qb}")
                    for qb in range(NQB)] for p in range(2)]
            vvt = [sb.tile([128, HPC * VW], BF16, tag=f"vv{jc}", name=f"vv{jc}")
                   for jc in range(NJC)]
            oTn = [[sb.tile([64, QB], F32R, tag=f"oTn{h}_{qb}", name=f"oTn{h}_{qb}")
                    for qb in range(NQB)] for h in range(HPC)]
            ones32 = sb.tile([128, 1], F32, tag="ones32", name="ones32")
            nc.vector.memset(ones32[:], 1.0)

            # ---- proj group emitters (each ~1-2k PE cycles + a DVE copy) ----
            def kq_group(dst, wsb, p, col0):
                # dst[:, :] = (w pair-slice).T @ x[:, col0:col0+512]
                ps = pp.tile([128, 512], F32, tag="pg", bufs=2, name="ps")
                for d in range(4):
                    nc.tensor.matmul(
                        ps[:], wsb[d][:, p * 128:(p + 1) * 128],
                        xt[d][:, col0:col0 + 512],
                        start=(d == 0), stop=(d == 3))
                nc.vector.tensor_copy(out=dst[:], in_=ps[:])

            def vv_group(jc):
                psv = pp.tile([128, 512], F32, tag="pg", bufs=2, name="psv")
                for d in range(4):
                    nc.tensor.matmul(
                        psv[:, 0:DQ], xt[d][:, jc * 128:(jc + 1) * 128],
                        wv[d][:, :], start=(d == 0), stop=(d == 3))
                vv_ones = vvt[jc][:, :].rearrange(
                    "p (g w) -> p g w", w=VW)[:, :, HD:HD + 1]
                nc.vector.tensor_copy(
                    out=vv_ones, in_=ones32[:].to_broadcast((128, HPC, 1)))
                for h in range(HPC):
                    nc.vector.tensor_copy(
                        out=vvt[jc][:, VW * h: VW * h + HD],
                        in_=psv[:, HD * h: HD * (h + 1)])

            def outproj_group(m, qb):
                # outT[m-chunk, qb-block] = sum_h woT[h-dims, m-chunk].T @ oTn
                po = pp.tile([128, 512], F32, tag="pg", bufs=2, name="po")
                for kc in range(4):
                    nc.tensor.matmul(
                        po[:], wo[kc][:, m * 128:(m + 1) * 128],
                        oTn[kc][qb][:], start=(kc == 0), stop=(kc == 3))
                ob = sb.tile([128, 512], F32, tag="ob", bufs=4, name="ob")
                nc.vector.tensor_copy(out=ob[:], in_=po[:])
                nc.sync.dma_start(out=outTc[m, qb], in_=ob[:])

            # ---- explicit filler schedule: unit u = qb*2 + p; slots[u][jc]
            # is a list of closures emitted right after attnv(jc-1), i.e. in
            # the PE's ACT-bound slack.  Placement respects (a) emission
            # before consumption, (b) input-DMA landing times (so a stalled
            # filler never blocks the FIFO PE queue ahead of scores).
            slots = [dict() for _ in range(2 * NQB)]

            def put(u, jc, fn):
                slots[u].setdefault(jc, []).append(fn)

            # unit 0 = (qb0, p0): stream in remaining vv + kT chunks
            for k in range(2, NJC):
                put(0, k - 2, lambda jc=k: vv_group(jc))
            put(0, 3, lambda: kq_group(kTt[0][1], wk, 0, 512))
            put(0, 5, lambda: kq_group(kTt[0][2], wk, 0, 1024))
            put(0, 9, lambda: kq_group(kTt[0][3], wk, 0, 1536))
            put(0, 11, lambda: kq_group(kTt[1][0], wk, 1, 0))
            put(0, 13, lambda: kq_group(kTt[1][1], wk, 1, 512))
            put(0, 15, lambda: kq_group(qTt[1][0], wq, 1, 0))
            # unit 1 = (qb0, p1): its own late kT chunks + next qT
            put(1, 1, lambda: kq_group(kTt[1][2], wk, 1, 1024))
            put(1, 3, lambda: kq_group(kTt[1][3], wk, 1, 1536))
            put(1, 9, lambda: kq_group(qTt[0][1], wq, 0, 512))
            for qb in range(1, NQB):
                u0 = 2 * qb
                # outproj of the previous qblock goes in the ODD unit (a
                # full unit, ~18us, after the epilogues that produce its
                # oTn inputs -- their DMA round-trip chain must not stall
                # the PE FIFO in front of scores).
                for m, j in enumerate((5, 7, 11, 13)):
                    put(u0 + 1, j, lambda m=m, qb=qb: outproj_group(m, qb - 1))
                put(u0, 9, lambda qb=qb: kq_group(qTt[1][qb], wq, 1, qb * 512))
                if qb + 1 < NQB:
                    put(u0 + 1, 9, lambda qb=qb: kq_group(
                        qTt[0][qb + 1], wq, 0, (qb + 1) * 512))

            # ---- phase P: minimal prefix before unit (qb0, p0) ----
            with nc.named_scope("proj"):
                kq_group(kTt[0][0], wk, 0, 0)
                kq_group(qTt[0][0], wq, 0, 0)
                vv_group(0)
                vv_group(1)

            # ---- attention epilogue: softmax denominators, all on-chip.
            # otu row 64 holds sums[q].  Stage A transposes it to [128, 4]
            # via 4 tiny matmuls and takes the reciprocal on the DVE (the
            # [128, p] layout gives it lanes).  Stage B broadcasts it back to
            # [64, 512] in one pass: lhsT = recT column broadcast (stride-0)
            # against the identity, so psC[d, 128j+n] = 1/sums[128j+n].
            def epi_A(otu, recT):
                psA = pp.tile([128, 512], F32, tag="pg", bufs=2, name="psA")
                for j in range(4):
                    nc.tensor.matmul(
                        psA[:, j:j + 1], otu[64:65, 128 * j:128 * (j + 1)],
                        ones32[64:65, 0:1], start=True, stop=True)
                nc.vector.reciprocal(recT[:], psA[:, 0:4])

            def epi_B(h, qb, otu, recT):
                psC = pp.tile([128, 512], F32, tag="pg", bufs=2, name="psC")
                for j in range(4):
                    nc.tensor.matmul(
                        psC[0:64, 128 * j:128 * (j + 1)],
                        recT[:, j:j + 1].to_broadcast((128, 64)),
                        ident[:, :], start=True, stop=True)
                nc.vector.tensor_mul(
                    out=oTn[h][qb][:], in0=otu[0:64, :], in1=psC[0:64, :])

            with nc.named_scope("attn"):
                for u in range(2 * NQB):
                    qb, p = u // 2, u % 2
                    op0 = pp.tile([65, QB], F32, tag="op0", bufs=1, name="op0")
                    op1 = pp.tile([65, QB], F32, tag="op1", bufs=1, name="op1")
                    ats = [None] * 16
                    # software-pipelined: attnv lags scores/exp by one jc so
                    # the FIFO PE queue never parks scores behind an
                    # exp-dependent matmul.
                    for jc in range(16):
                        sp = pp.tile([128, 1024], F32, tag="sp", bufs=2,
                                     name="sp")
                        sc, c0 = jc // 4, (jc % 4) * 128
                        for e in range(2):
                            nc.tensor.matmul(
                                sp[:, e * 512:(e + 1) * 512],
                                kTt[p][sc][64 * e:64 * e + 64, c0:c0 + 128],
                                qTt[p][qb][64 * e:64 * e + 64, :],
                                start=True, stop=True,
                                tile_position=(64 * e, 0))
                        at = sb.tile([128, 1024], BF16, tag="at", bufs=3,
                                     name="at")
                        nc.scalar.activation(at[:], sp[:], EXP)
                        ats[jc] = at

                        def attnv(j):
                            for e, op_t in ((0, op0), (1, op1)):
                                nc.tensor.matmul(
                                    op_t[:],
                                    vvt[j][:, VW * (2 * p + e):
                                           VW * (2 * p + e) + VW],
                                    ats[j][:, e * 512:(e + 1) * 512],
                                    start=(j == 0), stop=(j == 15))
                        if jc > 0:
                            attnv(jc - 1)
                        for fn in slots[u].get(jc, ()):
                            fn()
                    attnv(15)
                    # drain op psum now (DVE only); the PE stages run as
                    # next-unit slot fillers so their DVE-dependency waits
                    # never bubble the FIFO PE queue in front of scores.
                    epis = []
                    for e, op_t in ((0, op0), (1, op1)):
                        h = 2 * p + e
                        otu = sb.tile([65, QB], F32, tag="otu", bufs=2,
                                      name="otu")
                        nc.vector.tensor_copy(out=otu[:], in_=op_t[:])
                        recT = sb.tile([128, 4], F32, tag="recT", bufs=2,
                                       name="recT")
                        epis.append((h, otu, recT))
                    if u + 1 < 2 * NQB:
                        for e, (h, otu, recT) in enumerate(epis):
                            put(u + 1, 1 + e, lambda otu=otu, recT=recT:
                                epi_A(otu, recT))
                            put(u + 1, 3 + e, lambda h=h, qb=qb, otu=otu,
                                recT=recT: epi_B(h, qb, otu, recT))
                    else:
                        for h, otu, recT in epis:
                            epi_A(otu, recT)
                            epi_B(h, qb, otu, recT)

            with nc.named_scope("outproj"):
                for m in range(4):
                    outproj_group(m, NQB - 1)

            psum.__exit__(None, None, None)

    nc.compile()
    return nc


def _get_nc():
    if "nc" not in _cache:
        _cache["nc"] = _build_nc()
    return _cache["nc"]


def _in_maps(x, w_qkv, w_out):
    x = np.asarray(x, dtype=np.float32)
    w_qkv = np.asarray(w_qkv, dtype=np.float32)
    w_out = np.asarray(w_out, dtype=np.float32)
    maps = []
    for c in range(NCORES):
        b, qh = c // 2, c % 2
        r0 = qh * DQ
        xT = x[b].T  # [D, S]
        xTc = np.ascontiguousarray(
            xT.reshape(D, 4, 512).transpose(1, 0, 2))  # [4, D, 512]
        maps.append({
            "xTc": xTc,
            "identT": np.eye(128, dtype=np.float32),
            "wqT": np.ascontiguousarray(w_qkv[r0:r0 + DQ].T),
            "wkT": np.ascontiguousarray(w_qkv[D + r0:D + r0 + DQ].T),
            "wvT": np.ascontiguousarray(w_qkv[2 * D + r0:2 * D + r0 + DQ].T),
            "woT": np.ascontiguousarray(w_out[:, r0:r0 + DQ].T),
        })
    return maps


def _gather(results):
    out = np.empty((B, S, D), np.float32)
    for b in range(B):
        acc = results[2 * b]["outTc"] + results[2 * b + 1]["outTc"]
        # [4(m), NQB, 128, 512] -> outT [D, S] -> out [S, D]
        outT = acc.transpose(0, 2, 1, 3).reshape(D, S)
        out[b] = outT.T
    return out


def run(x, w_qkv, w_out, trace=False):
    from concourse.bass_utils import run_bass_kernel_spmd

    nc = _get_nc()
    res = run_bass_kernel_spmd(
        nc, _in_maps(x, w_qkv, w_out), core_ids=list(range(NCORES)), trace=trace,
    )
    return _gather(res.results), res


def kernel(x, w_qkv, w_out):
    out, _ = run(x, w_qkv, w_out)
    return out
